# revision 16
# baseline (speedup 1.0000x reference)
"""ChebyKANLinear Trainium2 kernel.

Math: y[b,o] = (1/I) * sum_{i,d} T_d(c[b,i]) * W[i,o,d],  c = tanh(x)
with Chebyshev T_0=1, T_1=c, T_2=2c^2-1, T_3=4c^3-3c.
(The reference also clips c to [-1+1e-7, 1-1e-7] before arccos; in the
monomial form below the bound is numerically irrelevant — |tanh|max for this
input distribution is 0.99992, far below it — so the clip is dropped.)

Re-expressed in the monomial basis (exact linear recombination, folded into
the weights on the host):
    y = bias + c @ V1 + c^2 @ V2 + c^3 @ V3
    V1 = (W1 - 3*W3)/I, V2 = 2*W2/I, V3 = 4*W3/I, bias_o = sum_i (W0 - W2)[i,o]/I

Sharding: 2D — batch into 4 shards x output_dim into 2 shards across the 8
NeuronCores. Per core the matmuls are computed TRANSPOSED,
    yT[o, b] = sum_k  V_k[i, o].T @ (c^k)[i, b]
so each core runs only 6 fp32 matmuls of [K=128, M=128, N=512] (N=512 is the
fp32 moving-operand max — fewest PE passes for this contraction), and the
bias becomes a per-partition scalar fused into the PSUM->SBUF copy
(vector.tensor_scalar_add) instead of costing extra matmuls.

Perf notes baked in from trace analysis:
- All of V plus the bias column ride ONE wide-row dma_start ([128, 769] ->
  3KB/partition rows); narrow-row DMAs measured ~3x slower per byte.
- x rides two dma_starts on the other HWDGE queue (sync/SP).
- Two real-shaped (K=128, N=512) warmup matmuls on memset tiles run during
  the DMA phase so the PE HAM clock-gate (1.2 -> 2.4 GHz) opens right as the
  real accumulation chain peaks.
- Output is written as two half DMAs on the two queues to overlap the
  PSUM->SBUF bias-add with the store.
"""

from contextlib import ExitStack

import numpy as np

import concourse.bass as bass
import concourse.tile as tile
from concourse import bacc, mybir
from concourse.bass_utils import run_bass_kernel_spmd

N_CORES = 8
B, I, O, D = 2048, 256, 256, 4
RB, SO = 4, 2  # batch shards x output shards
BL = B // RB  # 512 batch rows per core
OL = O // SO  # 128 output cols per core
F32 = mybir.dt.float32

_cache = {}


def _build_program():
    nc = bacc.Bacc("TRN2", target_bir_lowering=False, debug=False, num_devices=N_CORES)

    # [i_half, i_in_half, b_local]  (x slice pre-transposed on host)
    xt_d = nc.dram_tensor("xt", [2, 128, BL], F32, kind="ExternalInput")
    # packed weights: col (ih*3+d)*OL + o holds V[d, ih*128+i, o]; col 768 = bias
    vb_d = nc.dram_tensor("vb", [128, 6 * OL + 1], F32, kind="ExternalInput")
    # transposed output [o_local, b_local]
    y_d = nc.dram_tensor("y", [OL, BL], F32, kind="ExternalOutput")

    with tile.TileContext(nc) as tc, ExitStack() as ctx:
        pool = ctx.enter_context(tc.tile_pool(name="main", bufs=1))
        psum = ctx.enter_context(
            tc.tile_pool(name="psum", bufs=1, space=bass.MemorySpace.PSUM)
        )

        # PE warmup operands (DVE is idle this early; values are irrelevant)
        wu_w = pool.tile([128, 128], F32, tag="wu_w")
        nc.vector.memset(wu_w[:], 1.0)
        wu_r = pool.tile([128, 512], F32, tag="wu_r")
        nc.vector.memset(wu_r[:], 1.0)

        # One dma_start per tensor; x pair on the sync queue (the scalar
        # HWDGE queue measured ~1us slower to first byte), packed V+bias on
        # scalar. Splitting tensors across queues and other rebalances all
        # measured slower (per-dma fixed cost + queue startup).
        # tiny ring-prewarm DMAs eat each queue's first-byte latency before
        # the real transfers queue up behind them
        pre_s = pool.tile([1, 16], F32, tag="pre_s")
        nc.sync.dma_start(pre_s[:], xt_d[0, 0:1, 0:16])
        pre_a = pool.tile([1, 16], F32, tag="pre_a")
        nc.scalar.dma_start(pre_a[:], vb_d[0:1, 0:16])
        vb = pool.tile([128, 6 * OL + 1], F32, tag="vb")
        nc.scalar.dma_start(vb[:], vb_d[:])
        xt = {}
        for ih in range(2):
            xt[ih] = pool.tile([128, BL], F32, tag=f"xt{ih}", name=f"xt{ih}")
        nc.sync.dma_start(xt[0][:], xt_d[0])
        nc.sync.dma_start(xt[1][:], xt_d[1])

        # Two warmup matmuls: dense K=128 N=512 so the HAM clock-gate sees
        # real PE activity; they end right as the real chain starts (a gap
        # would reset the HAM busy-window progress — measured).
        wu_acc = psum.tile([128, 512], F32, tag="wu_acc")
        for _ in range(2):
            nc.tensor.matmul(wu_acc[:], wu_w[:], wu_r[:], start=True, stop=True)

        # basis: c = tanh(xT) on ACT, c^2/c^3 on DVE
        basis = {}
        for ih in range(2):
            c = pool.tile([128, BL], F32, tag=f"c{ih}")
            nc.scalar.activation(c[:], xt[ih][:], mybir.ActivationFunctionType.Tanh)
            basis[(0, ih)] = c
        for ih in range(2):
            c2 = pool.tile([128, BL], F32, tag=f"c2{ih}")
            nc.vector.tensor_mul(c2[:], basis[(0, ih)][:], basis[(0, ih)][:])
            basis[(1, ih)] = c2
        for ih in range(2):
            c3 = pool.tile([128, BL], F32, tag=f"c3{ih}")
            nc.vector.tensor_mul(c3[:], basis[(1, ih)][:], basis[(0, ih)][:])
            basis[(2, ih)] = c3

        # yT[o, b] accumulation: 6 matmuls alternating between TWO PSUM
        # banks (ih=0 -> acc_a, ih=1 -> acc_b) so consecutive accumulating
        # passes don't serialize on one bank; merged + bias in one fused
        # DVE op per half: (acc_a + bias) + acc_b.
        acc_a = psum.tile([128, BL], F32, tag="acc_a")
        acc_b = psum.tile([128, BL], F32, tag="acc_b")
        accs = {0: acc_a, 1: acc_b}
        mm_order = [(0, 0), (0, 1), (1, 0), (1, 1), (2, 0), (2, 1)]
        for n, (d, ih) in enumerate(mm_order):
            col = (ih * 3 + d) * OL
            nc.tensor.matmul(
                accs[ih][:OL, :],
                vb[:, col : col + OL],
                basis[(d, ih)][:],
                start=(d == 0),
                stop=(d == 2),
            )

        # DVE can read only ONE PSUM operand per op: pre-merge acc_a + bias
        # into SBUF (overlaps the final acc_b matmul), then y = tmp + acc_b.
        bias_col = vb[:, 6 * OL : 6 * OL + 1]
        tmp_sb = pool.tile([OL, BL], F32, tag="tmp_sb")
        y_sb = pool.tile([OL, BL], F32, tag="y_sb")
        half = BL // 2
        nc.vector.tensor_scalar_add(tmp_sb[:, :half], acc_a[:OL, :half], bias_col)
        nc.vector.tensor_scalar_add(tmp_sb[:, half:], acc_a[:OL, half:], bias_col)
        nc.vector.tensor_tensor(
            y_sb[:, :half], acc_b[:OL, :half], tmp_sb[:, :half], mybir.AluOpType.add
        )
        nc.sync.dma_start(y_d[:, :half], y_sb[:, :half])
        nc.vector.tensor_tensor(
            y_sb[:, half:], acc_b[:OL, half:], tmp_sb[:, half:], mybir.AluOpType.add
        )
        nc.scalar.dma_start(y_d[:, half:], y_sb[:, half:])

    nc.compile()
    return nc


def _get_program():
    if "nc" not in _cache:
        _cache["nc"] = _build_program()
    return _cache["nc"]


def _make_in_maps(x, cheby_coeffs):
    x = np.ascontiguousarray(x, dtype=np.float32)
    W = np.ascontiguousarray(cheby_coeffs, dtype=np.float32)
    assert x.shape == (B, I) and W.shape == (I, O, D)

    inv_i = np.float32(1.0 / I)
    V = np.stack(
        [
            W[:, :, 1] - 3.0 * W[:, :, 3],
            2.0 * W[:, :, 2],
            4.0 * W[:, :, 3],
        ]
    ).astype(np.float32) * inv_i  # [3, I, O]
    bias_full = (W[:, :, 0] - W[:, :, 2]).sum(axis=0, dtype=np.float32) * inv_i  # [O]

    xt_shards = []
    for rb in range(RB):
        xs = x[rb * BL : (rb + 1) * BL, :]  # [BL, I]
        xt_shards.append(np.ascontiguousarray(xs.T).reshape(2, 128, BL))
    vb_shards = []
    for so in range(SO):
        vb = np.empty((128, 6 * OL + 1), dtype=np.float32)
        for ih in range(2):
            for d in range(3):
                col = (ih * 3 + d) * OL
                # vb[i, col+o] = V[d, ih*128+i, so*OL+o]
                vb[:, col : col + OL] = V[
                    d, ih * 128 : (ih + 1) * 128, so * OL : (so + 1) * OL
                ]
        vb[:, 6 * OL] = bias_full[so * OL : (so + 1) * OL]
        vb_shards.append(vb)
    in_maps = []
    for c_id in range(N_CORES):
        rb, so = divmod(c_id, SO)
        in_maps.append({"xt": xt_shards[rb], "vb": vb_shards[so]})
    return in_maps


def kernel(x, cheby_coeffs):
    nc = _get_program()
    in_maps = _make_in_maps(x, cheby_coeffs)
    res = run_bass_kernel_spmd(nc, in_maps, list(range(N_CORES)))
    y = np.empty((B, O), dtype=np.float32)
    for c_id in range(N_CORES):
        rb, so = divmod(c_id, SO)
        y[rb * BL : (rb + 1) * BL, so * OL : (so + 1) * OL] = res.results[c_id]["y"].T
    return y


# revision 20
# speedup vs baseline: 1.1226x; 1.1226x over previous
"""ChebyKANLinear Trainium2 kernel.

Math: y[b,o] = (1/I) * sum_{i,d} T_d(c[b,i]) * W[i,o,d],  c = tanh(x)
with Chebyshev T_0=1, T_1=c, T_2=2c^2-1, T_3=4c^3-3c.
(The reference also clips c to [-1+1e-7, 1-1e-7] before arccos; in the
monomial form below the bound is numerically irrelevant — |tanh|max for this
input distribution is 0.99992, far below it — so the clip is dropped.)

Re-expressed in the monomial basis (exact linear recombination, folded into
the weights on the host):
    y = bias + c @ V1 + c^2 @ V2 + c^3 @ V3
    V1 = (W1 - 3*W3)/I, V2 = 2*W2/I, V3 = 4*W3/I, bias_o = sum_i (W0 - W2)[i,o]/I

Sharding: 2D — batch into 4 shards x output_dim into 2 shards across the 8
NeuronCores. Per core the matmuls are computed TRANSPOSED,
    yT[o, b] = sum_k  V_k[i, o].T @ (c^k)[i, b]
so each core runs only 6 fp32 matmuls of [K=128, M=128, N=512] (N=512 is the
fp32 moving-operand max — fewest PE passes for this contraction), and the
bias becomes a per-partition scalar fused into the PSUM->SBUF copy
(vector.tensor_scalar_add) instead of costing extra matmuls.

Perf notes baked in from trace analysis:
- All of V plus the bias column ride ONE wide-row dma_start ([128, 769] ->
  3KB/partition rows); narrow-row DMAs measured ~3x slower per byte.
- x rides two dma_starts on the other HWDGE queue (sync/SP).
- Two real-shaped (K=128, N=512) warmup matmuls on memset tiles run during
  the DMA phase so the PE HAM clock-gate (1.2 -> 2.4 GHz) opens right as the
  real accumulation chain peaks.
- Output is written as two half DMAs on the two queues to overlap the
  PSUM->SBUF bias-add with the store.
"""

from contextlib import ExitStack

import numpy as np

import concourse.bass as bass
import concourse.tile as tile
from concourse import bacc, mybir
from concourse.bass_utils import run_bass_kernel_spmd

N_CORES = 8
B, I, O, D = 2048, 256, 256, 4
RB, SO = 4, 2  # batch shards x output shards
BL = B // RB  # 512 batch rows per core
OL = O // SO  # 128 output cols per core
F32 = mybir.dt.float32

_cache = {}


def _build_program():
    nc = bacc.Bacc("TRN2", target_bir_lowering=False, debug=False, num_devices=N_CORES)

    # [i_half, i_in_half, b_local]  (x slice pre-transposed on host)
    xt_d = nc.dram_tensor("xt", [2, 128, BL], F32, kind="ExternalInput")
    # packed weights: col (ih*3+d)*OL + o holds V[d, ih*128+i, o]; col 768 = bias
    vb_d = nc.dram_tensor("vb", [128, 6 * OL + 1], F32, kind="ExternalInput")
    # transposed output [o_local, b_local]
    y_d = nc.dram_tensor("y", [OL, BL], F32, kind="ExternalOutput")

    with tile.TileContext(nc) as tc, ExitStack() as ctx:
        pool = ctx.enter_context(tc.tile_pool(name="main", bufs=1))
        psum = ctx.enter_context(
            tc.tile_pool(name="psum", bufs=1, space=bass.MemorySpace.PSUM)
        )

        # PE warmup operands (DVE is idle this early; values are irrelevant)
        wu_w = pool.tile([128, 128], F32, tag="wu_w")
        nc.vector.memset(wu_w[:], 1.0)
        wu_r = pool.tile([128, 512], F32, tag="wu_r")
        nc.vector.memset(wu_r[:], 1.0)

        # One dma_start per tensor; x pair on the sync queue (the scalar
        # HWDGE queue measured ~1us slower to first byte), packed V+bias on
        # scalar. Splitting tensors across queues and other rebalances all
        # measured slower (per-dma fixed cost + queue startup).
        vb = pool.tile([128, 6 * OL + 1], F32, tag="vb")
        nc.scalar.dma_start(vb[:], vb_d[:])
        xt = {}
        for ih in range(2):
            xt[ih] = pool.tile([128, BL], F32, tag=f"xt{ih}", name=f"xt{ih}")
        nc.sync.dma_start(xt[0][:], xt_d[0])
        nc.sync.dma_start(xt[1][:], xt_d[1])

        # Two warmup matmuls: dense K=128 N=512 so the HAM clock-gate sees
        # real PE activity; they end right as the real chain starts (a gap
        # would reset the HAM busy-window progress — measured).
        wu_acc = psum.tile([128, 512], F32, tag="wu_acc")
        for _ in range(2):
            nc.tensor.matmul(wu_acc[:], wu_w[:], wu_r[:], start=True, stop=True)

        # basis: c = tanh(xT) on ACT, c^2/c^3 on DVE
        basis = {}
        for ih in range(2):
            c = pool.tile([128, BL], F32, tag=f"c{ih}")
            nc.scalar.activation(c[:], xt[ih][:], mybir.ActivationFunctionType.Tanh)
            basis[(0, ih)] = c
        for ih in range(2):
            c2 = pool.tile([128, BL], F32, tag=f"c2{ih}")
            nc.vector.tensor_mul(c2[:], basis[(0, ih)][:], basis[(0, ih)][:])
            basis[(1, ih)] = c2
        for ih in range(2):
            c3 = pool.tile([128, BL], F32, tag=f"c3{ih}")
            nc.vector.tensor_mul(c3[:], basis[(1, ih)][:], basis[(0, ih)][:])
            basis[(2, ih)] = c3

        # yT[o, b] accumulation: 6 matmuls alternating between TWO PSUM
        # banks (ih=0 -> acc_a, ih=1 -> acc_b) so consecutive accumulating
        # passes don't serialize on one bank; merged + bias in one fused
        # DVE op per half: (acc_a + bias) + acc_b.
        acc_a = psum.tile([128, BL], F32, tag="acc_a")
        acc_b = psum.tile([128, BL], F32, tag="acc_b")
        accs = {0: acc_a, 1: acc_b}
        mm_order = [(0, 0), (0, 1), (1, 0), (1, 1), (2, 0), (2, 1)]
        for n, (d, ih) in enumerate(mm_order):
            col = (ih * 3 + d) * OL
            nc.tensor.matmul(
                accs[ih][:OL, :],
                vb[:, col : col + OL],
                basis[(d, ih)][:],
                start=(d == 0),
                stop=(d == 2),
            )

        # DVE can read only ONE PSUM operand per op: pre-merge acc_a + bias
        # into SBUF (overlaps the final acc_b matmul), then y = tmp + acc_b.
        bias_col = vb[:, 6 * OL : 6 * OL + 1]
        tmp_sb = pool.tile([OL, BL], F32, tag="tmp_sb")
        y_sb = pool.tile([OL, BL], F32, tag="y_sb")
        half = BL // 2
        nc.vector.tensor_scalar_add(tmp_sb[:, :half], acc_a[:OL, :half], bias_col)
        nc.vector.tensor_scalar_add(tmp_sb[:, half:], acc_a[:OL, half:], bias_col)
        q = BL // 4
        for k in range(4):
            s = slice(k * q, (k + 1) * q)
            nc.vector.tensor_tensor(
                y_sb[:, s], acc_b[:OL, s], tmp_sb[:, s], mybir.AluOpType.add
            )
            (nc.sync if k % 2 == 0 else nc.scalar).dma_start(y_d[:, s], y_sb[:, s])

    nc.compile()
    return nc


def _get_program():
    if "nc" not in _cache:
        _cache["nc"] = _build_program()
    return _cache["nc"]


def _make_in_maps(x, cheby_coeffs):
    x = np.ascontiguousarray(x, dtype=np.float32)
    W = np.ascontiguousarray(cheby_coeffs, dtype=np.float32)
    assert x.shape == (B, I) and W.shape == (I, O, D)

    inv_i = np.float32(1.0 / I)
    V = np.stack(
        [
            W[:, :, 1] - 3.0 * W[:, :, 3],
            2.0 * W[:, :, 2],
            4.0 * W[:, :, 3],
        ]
    ).astype(np.float32) * inv_i  # [3, I, O]
    bias_full = (W[:, :, 0] - W[:, :, 2]).sum(axis=0, dtype=np.float32) * inv_i  # [O]

    xt_shards = []
    for rb in range(RB):
        xs = x[rb * BL : (rb + 1) * BL, :]  # [BL, I]
        xt_shards.append(np.ascontiguousarray(xs.T).reshape(2, 128, BL))
    vb_shards = []
    for so in range(SO):
        vb = np.empty((128, 6 * OL + 1), dtype=np.float32)
        for ih in range(2):
            for d in range(3):
                col = (ih * 3 + d) * OL
                # vb[i, col+o] = V[d, ih*128+i, so*OL+o]
                vb[:, col : col + OL] = V[
                    d, ih * 128 : (ih + 1) * 128, so * OL : (so + 1) * OL
                ]
        vb[:, 6 * OL] = bias_full[so * OL : (so + 1) * OL]
        vb_shards.append(vb)
    in_maps = []
    for c_id in range(N_CORES):
        rb, so = divmod(c_id, SO)
        in_maps.append({"xt": xt_shards[rb], "vb": vb_shards[so]})
    return in_maps


def kernel(x, cheby_coeffs):
    nc = _get_program()
    in_maps = _make_in_maps(x, cheby_coeffs)
    res = run_bass_kernel_spmd(nc, in_maps, list(range(N_CORES)))
    y = np.empty((B, O), dtype=np.float32)
    for c_id in range(N_CORES):
        rb, so = divmod(c_id, SO)
        y[rb * BL : (rb + 1) * BL, so * OL : (so + 1) * OL] = res.results[c_id]["y"].T
    return y
